# revision 77
# baseline (speedup 1.0000x reference)
"""Trainium2 Bass kernel for nn_MultiHeadAttention_7413113553038.

Sharding: 8 cores = (batch b in {0,1}) x (query block of 512). Each core
computes all 4 heads of attention for its 512 queries against the full 2048
keys of its batch, plus the output projection, residual add and LayerNorm for
its rows. No collectives needed.

Per-core strategy (v2):
  - X inputs and QKV weights shipped in fp8-e4m3, packed into few DMA
    transfers with >=1KB per-partition contiguous runs; all Q/K/V
    projections are single DoubleRow matmuls (contraction 256 = 2 fp8
    rows/cell), halving projection PE time and input DMA bytes.
  - Q^T/K^T in f16 [d, seq]; scores computed transposed per head pair with
    heads at partition bases 0/64 (concurrent PE row groups); exp on the
    scalar engine (scale=1/8 folds in 1/sqrt(d_k)) emits p in f16; the
    multiplicative Gaussian band tables E = exp(bias) are applied on the
    (otherwise idle) GPSIMD engine; PV accumulates over 16 k-chunks with V
    augmented by a ones-column so the softmax denominator Z lands in psum
    row 64 for free.
  - Epilogue: fc is split per head (row-group paired); 1/Z is applied to
    the fc output [q, 256] as a per-partition scalar (q is the partition
    dim there), so no Z broadcast matmuls are needed. Z rows are
    PE-transposed to columns and reciprocal'd once per group; residual add,
    bn_stats/bn_aggr LayerNorm fused per 128-row chunk.
  - Input DMAs spread over the queues in first-use order so the PE starts
    early and stays dense through the HAM warmup window.
"""

import numpy as np

N_HEADS = 4
D_K = 64
B = 2
S = 2048
F = 256
QB = 512  # queries per core
P = 128
KC = S // P  # 16 k-chunks
SIGMA_HS = (5.0, 10.0, 20.0, 40.0)
LN_EPS = 1e-5
N_CORES = 8
# per-head causal-bias band width (g >= ~1e-4): ceil(4.292 * sigma)
BAND = (22, 43, 86, 172)
E01_W = 192
E25_W = 304


_CACHE = {}


def _gauss_tables():
    """Compact multiplicative Gaussian-bias band tables E = exp(g) in fp16,
    transposed-score layout (delta = q - k = off_t + j - i, off_t = 256-128t).

    Only the diagonal band where g >= ~1e-4 matters, so the tables store just
    the band:
      e01 [4,128,192]: e01[h,i,m] = exp(g_h(m - i + 128)), k-chunk slots 0,1
      e25 [4,128,304]: e25[h,i,m] = exp(g_h(m - i)), slots 2..5
    g_h(d) = exp(-d^2 / (2 sigma_h^2)) for d >= 0 else 0.
    """
    i = np.arange(P, dtype=np.float64)[None, :, None]
    sig = np.asarray(SIGMA_HS, dtype=np.float64)[:, None, None]

    m01 = np.arange(E01_W, dtype=np.float64)[None, None, :]
    d01 = m01 - i + 128.0
    g01 = np.where(d01 >= 0, np.exp(-(d01 ** 2) / (2 * sig ** 2)), 0.0)

    m25 = np.arange(E25_W, dtype=np.float64)[None, None, :]
    d25 = m25 - i
    g25 = np.where(d25 >= 0, np.exp(-(d25 ** 2) / (2 * sig ** 2)), 0.0)
    return (
        np.exp(g01).astype(np.float16),
        np.exp(g25).astype(np.float16),
    )


def _build_program():
    import concourse.bass as bass  # noqa: F401
    import concourse.tile as tile
    from concourse import bacc, mybir
    from concourse.masks import make_identity

    f32 = mybir.dt.float32
    f16 = mybir.dt.float16
    f8e4 = mybir.dt.float8e4
    AF = mybir.ActivationFunctionType
    ALU = mybir.AluOpType
    DR = mybir.MatmulPerfMode.DoubleRow

    nc = bacc.Bacc("TRN2", target_bir_lowering=False, debug=False)

    # inputs pre-packed on the host into exact SBUF layouts, grouped into few
    # DMA transfers with large per-partition contiguous runs
    xqt = nc.dram_tensor("xqt", [P, 2, QB], f8e4, kind="ExternalInput").ap()
    wq8 = nc.dram_tensor("wq8", [P, 2, F], f8e4, kind="ExternalInput").ap()
    wk8 = nc.dram_tensor("wk8", [P, 2, F], f8e4, kind="ExternalInput").ap()
    wv8 = nc.dram_tensor("wv8", [P, 2, F], f8e4, kind="ExternalInput").ap()
    # xkv[nb][:, 0] = X_K^T block, [:, 1] = X_V^T block
    xkv = nc.dram_tensor("xkv", [4, P, 2, 2, 512], f8e4, kind="ExternalInput").ap()
    # wfr[:, 0:2] = W_fc, [:, 2:6] = residual rows
    wfr = nc.dram_tensor("wfr", [P, 6, F], f16, kind="ExternalInput").ap()
    e01 = nc.dram_tensor("e01", [P, N_HEADS, E01_W], f16, kind="ExternalInput").ap()
    e25 = nc.dram_tensor("e25", [P, N_HEADS, E25_W], f16, kind="ExternalInput").ap()
    out = nc.dram_tensor("out", [P, 4, F], f32, kind="ExternalOutput").ap()

    with tile.TileContext(nc) as tc:
        with (
            tc.tile_pool(name="wpool", bufs=1) as wpool,
            tc.tile_pool(name="xpool", bufs=1) as xpool,
            tc.tile_pool(name="proj", bufs=1) as proj,
            tc.tile_pool(name="mmps", bufs=2, space="PSUM") as mmps,
            tc.tile_pool(name="spsum", bufs=2, space="PSUM") as spsum,
            tc.tile_pool(name="cpsum", bufs=2, space="PSUM") as cpsum,
            tc.tile_pool(name="ptpool", bufs=3) as ptpool,
            tc.tile_pool(name="opool", bufs=4) as opool,
        ):
            # ---- input DMAs: 2 hardware rings (sync + scalar; act-table
            # loads don't block the scalar queue), first-use order; xkv0
            # split K/V into separate tiles so kc-0 scores start earlier ----
            xk0_sb = xpool.tile([P, 2, 512], f8e4, tag="xk0", name="xk0")
            xv0_sb = xpool.tile([P, 2, 512], f8e4, tag="xv0", name="xv0")
            xkv_b = [None] + [
                xpool.tile([P, 2, 2, 512], f8e4, tag=f"xkv{nb}", name=f"xkv{nb}")
                for nb in range(1, 4)
            ]
            # sync: wq, wk, xqt, e01, e25, xkv2, wfr
            wq_sb = wpool.tile([P, 2, F], f8e4, tag="wq")
            nc.sync.dma_start(wq_sb, wq8)
            wk_sb = wpool.tile([P, 2, F], f8e4, tag="wk")
            nc.sync.dma_start(wk_sb, wk8)
            xqt_sb = xpool.tile([P, 2, QB], f8e4, tag="xqt")
            nc.sync.dma_start(xqt_sb, xqt)
            e01_sb = wpool.tile([P, N_HEADS, E01_W], f16, tag="e01")
            nc.sync.dma_start(e01_sb, e01)
            e25_sb = wpool.tile([P, N_HEADS, E25_W], f16, tag="e25")
            nc.sync.dma_start(e25_sb, e25)
            nc.sync.dma_start(xkv_b[2], xkv[2])
            wfr_sb = wpool.tile([P, 6, F], f16, tag="wfr")
            nc.sync.dma_start(wfr_sb, wfr)

            # scalar: xk0, wv, xv0, xkv1, xkv3
            nc.scalar.dma_start(xk0_sb, xkv[0][:, 0])
            wv_sb = wpool.tile([P, 2, F], f8e4, tag="wv")
            nc.scalar.dma_start(wv_sb, wv8)
            nc.scalar.dma_start(xv0_sb, xkv[0][:, 1])
            nc.scalar.dma_start(xkv_b[1], xkv[1])
            nc.scalar.dma_start(xkv_b[3], xkv[3])
            wfc_sb = wfr_sb[:, 0:2, :]
            res_t = wfr_sb[:, 2:6, :]

            ident_f = wpool.tile([P, P], f32, tag="identf")
            make_identity(nc, ident_f)
            eps_t = wpool.tile([P, 1], f32, tag="eps")
            nc.vector.memset(eps_t, LN_EPS)
            dummy16 = wpool.tile([P, P], f16, tag="dm16")
            nc.vector.memset(dummy16, 0.0)

            # ---- persistent tiles ----
            qt_sb = proj.tile([P, 2, QB], f16, tag="qt")
            kt_b = [
                proj.tile([P, 2, 512], f16, tag=f"kt{nb}", name=f"kt{nb}")
                for nb in range(4)
            ]
            v_b = [
                proj.tile([P, 4, N_HEADS, 65], f16, tag=f"v{nb}", name=f"v{nb}")
                for nb in range(4)
            ]
            ctx_sb = proj.tile([P, 2, QB], f16, tag="ctx")
            ztmp_z = proj.tile([P, N_HEADS, QB], f32, tag="z")
            rz_t = proj.tile([P, 16], f32, tag="rz")
            xacc = proj.tile([P, 4, F], f32, tag="xacc")
            o_sb = proj.tile([P, 4, F], f32, tag="osb")

            # ---- projections (fp8 operands; USE_DR picks DoubleRow
            # single-matmul contraction-256 vs two accumulating matmuls) ----
            USE_DR = True

            def project_qt(g):
                ps = mmps.tile([P, 512], f32, tag="mm", name=f"psq{g}")
                nc.tensor.matmul(
                    ps, wq_sb[:, :, g * P:(g + 1) * P], xqt_sb,
                    start=True, stop=True, perf_mode=DR,
                )
                nc.vector.tensor_copy(qt_sb[:, g, :], ps)

            def project_kt(nb, g):
                src = xk0_sb if nb == 0 else xkv_b[nb][:, 0]
                ps = mmps.tile([P, 512], f32, tag="mm", name=f"psk{nb}{g}")
                nc.tensor.matmul(
                    ps, wk_sb[:, :, g * P:(g + 1) * P], src,
                    start=True, stop=True, perf_mode=DR,
                )
                nc.vector.tensor_copy(kt_b[nb][:, g, :], ps)

            def project_v(nb):
                src = xv0_sb if nb == 0 else xkv_b[nb][:, 1]
                for j in range(4):
                    ps = mmps.tile([P, 512], f32, tag="mm", name=f"psv{nb}{j}")
                    psv = ps[:, :F]
                    nc.tensor.matmul(
                        psv, src[:, :, j * P:(j + 1) * P], wv_sb,
                        start=True, stop=True, perf_mode=DR,
                    )
                    nc.vector.tensor_copy(
                        v_b[nb][:, j, :, 0:64],
                        psv.rearrange("p (h d) -> p h d", h=N_HEADS),
                    )
                nc.vector.memset(v_b[nb][:, :, :, 64:65], 1.0)

            # ---- attention ----
            def attn_sc(G, kc):
                """Scores + exp for one k-chunk of head pair G; returns pt."""
                ps = spsum.tile([P, 2 * QB], f32, tag="sc", name=f"sc{G[0]}_{kc}")
                for hi, h in enumerate(G):
                    g, po = h // 2, (h % 2) * 64
                    nc.tensor.matmul(
                        ps[:, hi * QB:(hi + 1) * QB],
                        kt_b[kc // 4][po:po + 64, g, (kc % 4) * P:(kc % 4 + 1) * P],
                        qt_sb[po:po + 64, g, :],
                        start=True,
                        stop=True,
                    )
                pt = ptpool.tile([P, 2 * QB], f16, tag="pt", name=f"pt{G[0]}_{kc}")
                nc.scalar.activation(pt, ps, AF.Exp, scale=0.125)
                return pt

            def attn_pv(G, ctxps, kc, pt):
                """Band multiply (GPSIMD) + PV accumulate for one k-chunk."""
                for hi, h in enumerate(G):
                    if kc <= 5:
                        off_t = 256 - 128 * kc
                        j0 = max(0, -off_t)
                        j1 = min(512, BAND[h] + 128 - off_t)
                        j1 = min(512, (j1 + 7) & ~7)
                        if j1 > j0:
                            if kc <= 1:
                                c0 = (128 - 128 * kc) + j0
                                esl = e01_sb[:, h, c0:c0 + (j1 - j0)]
                            else:
                                c0 = j0 - 128 * (kc - 2)
                                esl = e25_sb[:, h, c0:c0 + (j1 - j0)]
                            nc.vector.tensor_mul(
                                pt[:, hi * QB + j0:hi * QB + j1],
                                pt[:, hi * QB + j0:hi * QB + j1],
                                esl,
                            )
                    nc.tensor.matmul(
                        ctxps[hi][0:65, :],
                        v_b[kc // 4][:, kc % 4, h, 0:65],
                        pt[:, hi * QB:(hi + 1) * QB],
                        start=(kc == 0),
                        stop=(kc == KC - 1),
                    )

            def attn_kc(G, ctxps, kc):
                attn_pv(G, ctxps, kc, attn_sc(G, kc))

            # ---- per-group epilogue as drip-feedable steps ----
            def epilogue_steps(G, ctxps, tail=False):
                gg = G[0] // 2
                state = {}

                def s_copies():
                    # z rows first: they unblock the PE transposes, keeping
                    # the tensor engine busy (HAM warm) through the epilogue.
                    # At the tail the scalar engine is idle (exps done), so
                    # split the copies across scalar and vector.
                    for hi, h in enumerate(G):
                        if tail and hi == 0:
                            nc.scalar.copy(
                                ztmp_z[64:65, h, :], ctxps[hi][64:65, :]
                            )
                        else:
                            nc.vector.tensor_copy(
                                ztmp_z[64:65, h, :], ctxps[hi][64:65, :]
                            )
                    for hi, h in enumerate(G):
                        po = (h % 2) * 64
                        if tail and hi == 0:
                            nc.scalar.copy(
                                ctx_sb[po:po + 64, gg, :], ctxps[hi][0:64, :]
                            )
                        else:
                            nc.vector.tensor_copy(
                                ctx_sb[po:po + 64, gg, :], ctxps[hi][0:64, :]
                            )

                def s_transp(which=(0, 1)):
                    if "zt_g" not in state:
                        state["zt_g"] = mmps.tile(
                            [P, 512], f32, tag="mm", name=f"zt{gg}"
                        )
                    zt_g = state["zt_g"]
                    for hi in which:
                        h = G[hi]
                        for qc in range(4):
                            nc.tensor.transpose(
                                zt_g[:, hi * 4 + qc:hi * 4 + qc + 1],
                                ztmp_z[64:65, h, qc * P:(qc + 1) * P],
                                ident_f[64:65, 64:65],
                            )

                def s_recip():
                    nc.vector.tensor_copy(
                        rz_t[:, gg * 8:gg * 8 + 8], state["zt_g"][:, 0:8]
                    )
                    nc.vector.reciprocal(
                        rz_t[:, gg * 8:gg * 8 + 8], rz_t[:, gg * 8:gg * 8 + 8]
                    )

                def s_fc(qc):
                    # fc split per head (row groups 0/64 run concurrently);
                    # 1/Z applied to the [q, F] outputs as per-partition
                    # scalar. At the tail the score psum banks are free, so
                    # the fc pairs draw from them (deeper pipelining than the
                    # 2-buffer mm pool allows).
                    def emit():
                        pss = []
                        if tail:
                            ps2 = spsum.tile(
                                [P, 2 * QB], f32, tag="sc", name=f"pso{gg}{qc}"
                            )
                            pss = [ps2[:, 0:F], ps2[:, QB:QB + F]]
                        else:
                            pss = [
                                mmps.tile(
                                    [P, 512], f32, tag="mm",
                                    name=f"pso{gg}{qc}{hi}",
                                )[:, :F]
                                for hi in range(2)
                            ]
                        for hi in range(2):
                            po = hi * 64
                            nc.tensor.matmul(
                                pss[hi],
                                ctx_sb[po:po + 64, gg, qc * P:(qc + 1) * P],
                                wfc_sb[po:po + 64, gg, :],
                                start=True,
                                stop=True,
                            )
                        rz0 = rz_t[:, gg * 8 + qc:gg * 8 + qc + 1]
                        rz1 = rz_t[:, gg * 8 + 4 + qc:gg * 8 + 4 + qc + 1]
                        if gg == 0:
                            t0 = opool.tile([P, F], f32, tag="x", name=f"t0{qc}")
                            nc.vector.scalar_tensor_tensor(
                                t0, pss[0], rz0, res_t[:, qc, :],
                                op0=ALU.mult, op1=ALU.add,
                            )
                            nc.vector.scalar_tensor_tensor(
                                xacc[:, qc, :], pss[1], rz1, t0,
                                op0=ALU.mult, op1=ALU.add,
                            )
                        elif tail:
                            # scalar + gpsimd are idle at the tail: spread the
                            # per-qc chain so the vector engine stops binding
                            t0 = opool.tile([P, F], f32, tag="x", name=f"u0{qc}")
                            nc.scalar.mul(t0, pss[0], rz0)
                            x1 = opool.tile([P, F], f32, tag="x", name=f"v1{qc}")
                            nc.vector.scalar_tensor_tensor(
                                x1, pss[1], rz1, xacc[:, qc, :],
                                op0=ALU.mult, op1=ALU.add,
                            )
                            x_t = opool.tile([P, F], f32, tag="x", name=f"x{qc}")
                            nc.gpsimd.tensor_add(x_t, x1, t0)
                            st = opool.tile([P, 6], f32, tag="st", name=f"st{qc}")
                            nc.vector.bn_stats(st, x_t)
                            mv = opool.tile([P, 2], f32, tag="mv", name=f"mv{qc}")
                            nc.vector.bn_aggr(mv, st)
                            nc.scalar.activation(
                                mv[:, 1:2], mv[:, 1:2], AF.Sqrt,
                                bias=eps_t, scale=1.0,
                            )
                            nc.vector.reciprocal(mv[:, 1:2], mv[:, 1:2])
                            # negmb = -(mean * rstd); normalize on the scalar
                            # engine: out = Identity(x * rstd + negmb)
                            nc.vector.tensor_scalar(
                                mv[:, 0:1],
                                mv[:, 0:1],
                                mv[:, 1:2],
                                -1.0,
                                op0=ALU.mult,
                                op1=ALU.mult,
                            )
                            nc.scalar.activation(
                                o_sb[:, qc, :], x_t, AF.Identity,
                                bias=mv[:, 0:1], scale=mv[:, 1:2],
                            )
                            # ship each 128-query chunk as soon as it's done,
                            # alternating the two DMA rings
                            if qc % 2 == 0:
                                nc.sync.dma_start(
                                    out[:, qc:qc + 1, :], o_sb[:, qc:qc + 1, :]
                                )
                            else:
                                nc.scalar.dma_start(
                                    out[:, qc:qc + 1, :], o_sb[:, qc:qc + 1, :]
                                )
                    return emit

                return [s_copies, s_transp, s_recip,
                        s_fc(0), s_fc(1), s_fc(2), s_fc(3)]

            # ---- group 0: projections interleaved with its attention ----
            G0, G1 = (0, 1), (2, 3)
            ctxps0 = [
                cpsum.tile([P, QB], f32, tag="ctxp", name=f"ctxp{hh}")
                for hh in G0
            ]
            # HAM pre-warm: ~3.8us of dummy f16 matmuls in the input-DMA
            # shadow so the PE clock-gate opens (1.2 -> 2.4 GHz) before the
            # real stream begins. Results land in ctxps0[0] and are discarded
            # by the start=True of PV kc 0.
            for i in range(34):
                nc.tensor.matmul(
                    ctxps0[0][:, 0:P], dummy16, dummy16,
                    start=True, stop=True,
                )
            # head ordering: G0 (heads 0,1) only reads the g=0 halves of
            # Q^T/K^T and heads 0,1 of V, so only those are projected up
            # front; all g=1/h23 projections are dripped in later, ahead of
            # the G1 phase. This gets the first exps issued ~2us earlier.
            project_kt(0, 0)
            project_kt(0, 1)
            project_qt(0)
            project_qt(1)
            pt0 = attn_sc(G0, 0)
            pt1 = attn_sc(G0, 1)
            project_v(0)
            pt2 = attn_sc(G0, 2)
            attn_pv(G0, ctxps0, 0, pt0)
            pt3 = attn_sc(G0, 3)
            attn_pv(G0, ctxps0, 1, pt1)
            attn_pv(G0, ctxps0, 2, pt2)
            attn_pv(G0, ctxps0, 3, pt3)

            def proj(*specs):
                def emit():
                    for f, *a in specs:
                        f(*a)
                return emit
            proj_drip = {
                3: proj((project_kt, 1, 0), (project_kt, 1, 1),
                        (project_v, 1)),
                6: proj((project_kt, 2, 0), (project_kt, 2, 1)),
                7: proj((project_v, 2),),
                10: proj((project_kt, 3, 0), (project_kt, 3, 1)),
                11: proj((project_v, 3),),
            }
            proj_drip[3]()
            for kc in range(4, KC):
                attn_kc(G0, ctxps0, kc)
                if kc in proj_drip:
                    proj_drip[kc]()
            steps0 = epilogue_steps(G0, ctxps0)
            steps0[0]()  # ctx/z copies (DVE only, releases ctxps slots)

            # ---- group 1: attention with group-0 epilogue drip-fed in ----
            ctxps1 = [
                cpsum.tile([P, QB], f32, tag="ctxp", name=f"ctxp{hh}")
                for hh in G1
            ]
            drip = {1: lambda: steps0[1]((0,)), 2: lambda: steps0[1]((1,)),
                    4: steps0[2], 6: steps0[3], 8: steps0[4],
                    10: steps0[5], 12: steps0[6]}
            for kc in range(KC):
                attn_kc(G1, ctxps1, kc)
                if kc in drip:
                    drip[kc]()
            # keep the PE busy while the last exps drain so the HAM gate
            # stays open for the epilogue transposes + fc matmuls
            steps1 = epilogue_steps(G1, ctxps1, tail=True)
            steps1[0]()  # ctx/z copies
            warm_ps = spsum.tile([P, 2 * QB], f32, tag="sc", name="warmtail")
            for i in range(8):
                nc.tensor.matmul(
                    warm_ps[:, 0:P], dummy16, dummy16,
                    start=True, stop=True,
                )
            for step in steps1[1:]:
                step()

    nc.compile()
    return nc


def get_nc():
    if "nc" not in _CACHE:
        _CACHE["nc"] = _build_program()
    return _CACHE["nc"]


def make_in_maps(input_Q, input_K, input_V, W_Q, W_K, W_V, W_fc):
    import ml_dtypes

    f8 = ml_dtypes.float8_e4m3
    c16 = lambda a: np.ascontiguousarray(
        np.asarray(a, dtype=np.float32), dtype=np.float16
    )
    c8 = lambda a: np.clip(np.asarray(a, dtype=np.float32), -240, 240).astype(f8)
    # pack an [in, out]-style matrix to SBUF layout [p, c, out]
    pk_w = lambda w: np.asarray(w, np.float32).reshape(2, P, -1).transpose(1, 0, 2)
    # pack an activation block X [seq, F] to X^T SBUF layout [p, c, seq]
    pk_t = lambda x: c8(np.asarray(x, np.float32).T.reshape(2, P, -1).transpose(1, 0, 2))
    # pack a rolled key/value matrix [2048, F] to per-block X^T [nb, p, c, 512]
    pk_x = lambda x: np.asarray(x, np.float32).reshape(4, 512, 2, P).transpose(0, 3, 2, 1)
    e01t, e25t = _gauss_tables()
    e01 = np.ascontiguousarray(e01t.transpose(1, 0, 2))
    e25 = np.ascontiguousarray(e25t.transpose(1, 0, 2))
    e01_neutral = np.ones_like(e01)
    # unscaled W_Q; 1/sqrt(d_k) folded into the exp scale
    wq = c8(pk_w(W_Q))
    wk = c8(pk_w(W_K))
    wv8 = c8(pk_w(W_V))
    # wfr: [P, 6, F] f16 = W_fc (2 c-chunks) ++ residual (4 seq chunks)
    wfc_p = np.asarray(W_fc, np.float32).reshape(2, P, F).transpose(1, 0, 2)
    in_maps = []
    for c in range(N_CORES):
        b, qb = divmod(c, 4)
        q0 = qb * QB
        r = (q0 - 256) % S
        xq_blk = np.asarray(input_Q[b][q0:q0 + QB], np.float32)
        xk_rot = np.roll(np.asarray(input_K[b], np.float32), -r, axis=0)
        xv_rot = np.roll(np.asarray(input_V[b], np.float32), -r, axis=0)
        xkv = c8(np.stack([pk_x(xk_rot), pk_x(xv_rot)], axis=2))
        wfr = c16(np.concatenate(
            [wfc_p, xq_blk.reshape(4, P, F).transpose(1, 0, 2)], axis=1
        ))
        in_maps.append({
            "xqt": pk_t(xq_blk),
            "wq8": wq,
            "wk8": wk,
            "wv8": wv8,
            "xkv": xkv,
            "wfr": wfr,
            "e01": e01_neutral if q0 == 0 else e01,
            "e25": e25,
        })
    return in_maps


def kernel(input_Q, input_K, input_V, W_Q, W_K, W_V, W_fc, attn_mask=None):
    from concourse.bass_utils import run_bass_kernel_spmd

    nc = get_nc()
    in_maps = make_in_maps(input_Q, input_K, input_V, W_Q, W_K, W_V, W_fc)
    res = run_bass_kernel_spmd(nc, in_maps, core_ids=list(range(N_CORES)))
    out = np.empty((B, S, F), dtype=np.float32)
    for c in range(N_CORES):
        b, qb = divmod(c, 4)
        o = res.results[c]["out"]
        out[b, qb * QB:(qb + 1) * QB, :] = o.transpose(1, 0, 2).reshape(QB, F)
    return out


# revision 81
# speedup vs baseline: 1.0061x; 1.0061x over previous
"""Trainium2 Bass kernel for nn_MultiHeadAttention_7413113553038.

Sharding: 8 cores = (batch b in {0,1}) x (query block of 512). Each core
computes all 4 heads of attention for its 512 queries against the full 2048
keys of its batch, plus the output projection, residual add and LayerNorm for
its rows. No collectives needed.

Per-core strategy (v2):
  - X inputs and QKV weights shipped in fp8-e4m3, packed into few DMA
    transfers with >=1KB per-partition contiguous runs; all Q/K/V
    projections are single DoubleRow matmuls (contraction 256 = 2 fp8
    rows/cell), halving projection PE time and input DMA bytes.
  - Q^T/K^T in f16 [d, seq]; scores computed transposed per head pair with
    heads at partition bases 0/64 (concurrent PE row groups); exp on the
    scalar engine (scale=1/8 folds in 1/sqrt(d_k)) emits p in f16; the
    multiplicative Gaussian band tables E = exp(bias) are applied on the
    (otherwise idle) GPSIMD engine; PV accumulates over 16 k-chunks with V
    augmented by a ones-column so the softmax denominator Z lands in psum
    row 64 for free.
  - Epilogue: fc is split per head (row-group paired); 1/Z is applied to
    the fc output [q, 256] as a per-partition scalar (q is the partition
    dim there), so no Z broadcast matmuls are needed. Z rows are
    PE-transposed to columns and reciprocal'd once per group; residual add,
    bn_stats/bn_aggr LayerNorm fused per 128-row chunk.
  - Input DMAs spread over the queues in first-use order so the PE starts
    early and stays dense through the HAM warmup window.
"""

import numpy as np

N_HEADS = 4
D_K = 64
B = 2
S = 2048
F = 256
QB = 512  # queries per core
P = 128
KC = S // P  # 16 k-chunks
SIGMA_HS = (5.0, 10.0, 20.0, 40.0)
LN_EPS = 1e-5
N_CORES = 8
# per-head causal-bias band width (g >= ~1e-4): ceil(4.292 * sigma)
BAND = (22, 43, 86, 172)
E01_W = 192
E25_W = 304


_CACHE = {}


def _gauss_tables():
    """Compact multiplicative Gaussian-bias band tables E = exp(g) in fp16,
    transposed-score layout (delta = q - k = off_t + j - i, off_t = 256-128t).

    Only the diagonal band where g >= ~1e-4 matters, so the tables store just
    the band:
      e01 [4,128,192]: e01[h,i,m] = exp(g_h(m - i + 128)), k-chunk slots 0,1
      e25 [4,128,304]: e25[h,i,m] = exp(g_h(m - i)), slots 2..5
    g_h(d) = exp(-d^2 / (2 sigma_h^2)) for d >= 0 else 0.
    """
    i = np.arange(P, dtype=np.float64)[None, :, None]
    sig = np.asarray(SIGMA_HS, dtype=np.float64)[:, None, None]

    m01 = np.arange(E01_W, dtype=np.float64)[None, None, :]
    d01 = m01 - i + 128.0
    g01 = np.where(d01 >= 0, np.exp(-(d01 ** 2) / (2 * sig ** 2)), 0.0)

    m25 = np.arange(E25_W, dtype=np.float64)[None, None, :]
    d25 = m25 - i
    g25 = np.where(d25 >= 0, np.exp(-(d25 ** 2) / (2 * sig ** 2)), 0.0)
    return (
        np.exp(g01).astype(np.float16),
        np.exp(g25).astype(np.float16),
    )


def _build_program():
    import concourse.bass as bass  # noqa: F401
    import concourse.tile as tile
    from concourse import bacc, mybir
    from concourse.masks import make_identity

    f32 = mybir.dt.float32
    f16 = mybir.dt.float16
    f8e4 = mybir.dt.float8e4
    AF = mybir.ActivationFunctionType
    ALU = mybir.AluOpType
    DR = mybir.MatmulPerfMode.DoubleRow

    nc = bacc.Bacc("TRN2", target_bir_lowering=False, debug=False)

    # inputs pre-packed on the host into exact SBUF layouts, grouped into few
    # DMA transfers with large per-partition contiguous runs
    xqt = nc.dram_tensor("xqt", [P, 2, QB], f8e4, kind="ExternalInput").ap()
    wq8 = nc.dram_tensor("wq8", [P, 2, F], f8e4, kind="ExternalInput").ap()
    wk8 = nc.dram_tensor("wk8", [P, 2, F], f8e4, kind="ExternalInput").ap()
    wv8 = nc.dram_tensor("wv8", [P, 2, F], f8e4, kind="ExternalInput").ap()
    # xkv[nb][:, 0] = X_K^T block, [:, 1] = X_V^T block
    xkv = nc.dram_tensor("xkv", [4, P, 2, 2, 512], f8e4, kind="ExternalInput").ap()
    # wfr[:, 0:2] = W_fc, [:, 2:6] = residual rows
    wfr = nc.dram_tensor("wfr", [P, 6, F], f16, kind="ExternalInput").ap()
    e01 = nc.dram_tensor("e01", [P, N_HEADS, E01_W], f16, kind="ExternalInput").ap()
    e25 = nc.dram_tensor("e25", [P, N_HEADS, E25_W], f16, kind="ExternalInput").ap()
    out = nc.dram_tensor("out", [P, 4, F], f32, kind="ExternalOutput").ap()

    with tile.TileContext(nc) as tc:
        with (
            tc.tile_pool(name="wpool", bufs=1) as wpool,
            tc.tile_pool(name="xpool", bufs=1) as xpool,
            tc.tile_pool(name="proj", bufs=1) as proj,
            tc.tile_pool(name="mmps", bufs=2, space="PSUM") as mmps,
            tc.tile_pool(name="spsum", bufs=2, space="PSUM") as spsum,
            tc.tile_pool(name="cpsum", bufs=2, space="PSUM") as cpsum,
            tc.tile_pool(name="ptpool", bufs=3) as ptpool,
            tc.tile_pool(name="opool", bufs=4) as opool,
        ):
            # ---- input DMAs: 2 hardware rings (sync + scalar; act-table
            # loads don't block the scalar queue), first-use order; xkv0
            # split K/V into separate tiles so kc-0 scores start earlier ----
            xk0_sb = xpool.tile([P, 2, 512], f8e4, tag="xk0", name="xk0")
            xv0_sb = xpool.tile([P, 2, 512], f8e4, tag="xv0", name="xv0")
            xkv_b = [None] + [
                xpool.tile([P, 2, 2, 512], f8e4, tag=f"xkv{nb}", name=f"xkv{nb}")
                for nb in range(1, 4)
            ]
            # sync: wq, wk, xqt, e01, e25, xkv2, wfr
            wq_sb = wpool.tile([P, 2, F], f8e4, tag="wq")
            nc.sync.dma_start(wq_sb, wq8)
            wk_sb = wpool.tile([P, 2, F], f8e4, tag="wk")
            nc.sync.dma_start(wk_sb, wk8)
            xqt_sb = xpool.tile([P, 2, QB], f8e4, tag="xqt")
            nc.sync.dma_start(xqt_sb, xqt)
            e01_sb = wpool.tile([P, N_HEADS, E01_W], f16, tag="e01")
            nc.sync.dma_start(e01_sb, e01)
            e25_sb = wpool.tile([P, N_HEADS, E25_W], f16, tag="e25")
            nc.sync.dma_start(e25_sb, e25)
            nc.sync.dma_start(xkv_b[2], xkv[2])
            wfr_sb = wpool.tile([P, 6, F], f16, tag="wfr")
            nc.sync.dma_start(wfr_sb, wfr)

            # scalar: xk0, wv, xv0, xkv1, xkv3
            nc.scalar.dma_start(xk0_sb, xkv[0][:, 0])
            wv_sb = wpool.tile([P, 2, F], f8e4, tag="wv")
            nc.scalar.dma_start(wv_sb, wv8)
            nc.scalar.dma_start(xv0_sb, xkv[0][:, 1])
            nc.scalar.dma_start(xkv_b[1], xkv[1])
            nc.scalar.dma_start(xkv_b[3], xkv[3])
            wfc_sb = wfr_sb[:, 0:2, :]
            res_t = wfr_sb[:, 2:6, :]

            ident_f = wpool.tile([P, P], f32, tag="identf")
            make_identity(nc, ident_f)
            eps_t = wpool.tile([P, 1], f32, tag="eps")
            nc.vector.memset(eps_t, LN_EPS)
            dummy16 = wpool.tile([P, P], f16, tag="dm16")
            nc.vector.memset(dummy16, 0.0)

            # ---- persistent tiles ----
            qt_sb = proj.tile([P, 2, QB], f16, tag="qt")
            kt_b = [
                proj.tile([P, 2, 512], f16, tag=f"kt{nb}", name=f"kt{nb}")
                for nb in range(4)
            ]
            v_b = [
                proj.tile([P, 4, N_HEADS, 65], f16, tag=f"v{nb}", name=f"v{nb}")
                for nb in range(4)
            ]
            ctx_sb = proj.tile([P, 2, QB], f16, tag="ctx")
            ztmp_z = proj.tile([P, N_HEADS, QB], f32, tag="z")
            rz_t = proj.tile([P, 16], f32, tag="rz")
            xacc = proj.tile([P, 4, F], f32, tag="xacc")
            o_sb = proj.tile([P, 4, F], f32, tag="osb")

            # ---- projections (fp8 operands; USE_DR picks DoubleRow
            # single-matmul contraction-256 vs two accumulating matmuls) ----
            USE_DR = True

            def project_qt(g):
                ps = mmps.tile([P, 512], f32, tag="mm", name=f"psq{g}")
                nc.tensor.matmul(
                    ps, wq_sb[:, :, g * P:(g + 1) * P], xqt_sb,
                    start=True, stop=True, perf_mode=DR,
                )
                nc.vector.tensor_copy(qt_sb[:, g, :], ps)

            def project_kt(nb, g):
                src = xk0_sb if nb == 0 else xkv_b[nb][:, 0]
                ps = mmps.tile([P, 512], f32, tag="mm", name=f"psk{nb}{g}")
                nc.tensor.matmul(
                    ps, wk_sb[:, :, g * P:(g + 1) * P], src,
                    start=True, stop=True, perf_mode=DR,
                )
                nc.vector.tensor_copy(kt_b[nb][:, g, :], ps)

            def project_v(nb):
                src = xv0_sb if nb == 0 else xkv_b[nb][:, 1]
                for j in range(4):
                    ps = mmps.tile([P, 512], f32, tag="mm", name=f"psv{nb}{j}")
                    psv = ps[:, :F]
                    nc.tensor.matmul(
                        psv, src[:, :, j * P:(j + 1) * P], wv_sb,
                        start=True, stop=True, perf_mode=DR,
                    )
                    nc.vector.tensor_copy(
                        v_b[nb][:, j, :, 0:64],
                        psv.rearrange("p (h d) -> p h d", h=N_HEADS),
                    )
                nc.vector.memset(v_b[nb][:, :, :, 64:65], 1.0)

            # ---- attention ----
            def attn_sc(G, kc):
                """Scores + exp for one k-chunk of head pair G; returns pt."""
                ps = spsum.tile([P, 2 * QB], f32, tag="sc", name=f"sc{G[0]}_{kc}")
                for hi, h in enumerate(G):
                    g, po = h // 2, (h % 2) * 64
                    nc.tensor.matmul(
                        ps[:, hi * QB:(hi + 1) * QB],
                        kt_b[kc // 4][po:po + 64, g, (kc % 4) * P:(kc % 4 + 1) * P],
                        qt_sb[po:po + 64, g, :],
                        start=True,
                        stop=True,
                    )
                pt = ptpool.tile([P, 2 * QB], f16, tag="pt", name=f"pt{G[0]}_{kc}")
                nc.scalar.activation(pt, ps, AF.Exp, scale=0.125)
                return pt

            def attn_pv(G, ctxps, kc, pt):
                """Band multiply (GPSIMD) + PV accumulate for one k-chunk."""
                for hi, h in enumerate(G):
                    if kc <= 5:
                        off_t = 256 - 128 * kc
                        j0 = max(0, -off_t)
                        j1 = min(512, BAND[h] + 128 - off_t)
                        j1 = min(512, (j1 + 7) & ~7)
                        if j1 > j0:
                            if kc <= 1:
                                c0 = (128 - 128 * kc) + j0
                                esl = e01_sb[:, h, c0:c0 + (j1 - j0)]
                            else:
                                c0 = j0 - 128 * (kc - 2)
                                esl = e25_sb[:, h, c0:c0 + (j1 - j0)]
                            nc.vector.tensor_mul(
                                pt[:, hi * QB + j0:hi * QB + j1],
                                pt[:, hi * QB + j0:hi * QB + j1],
                                esl,
                            )
                    nc.tensor.matmul(
                        ctxps[hi][0:65, :],
                        v_b[kc // 4][:, kc % 4, h, 0:65],
                        pt[:, hi * QB:(hi + 1) * QB],
                        start=(kc == 0),
                        stop=(kc == KC - 1),
                    )

            def attn_kc(G, ctxps, kc):
                attn_pv(G, ctxps, kc, attn_sc(G, kc))

            # ---- per-group epilogue as drip-feedable steps ----
            def epilogue_steps(G, ctxps, tail=False):
                gg = G[0] // 2
                state = {}

                def s_copies():
                    # z rows first: they unblock the PE transposes, keeping
                    # the tensor engine busy (HAM warm) through the epilogue.
                    # At the tail the scalar engine is idle (exps done), so
                    # split the copies across scalar and vector.
                    for hi, h in enumerate(G):
                        if tail and hi == 0:
                            nc.scalar.copy(
                                ztmp_z[64:65, h, :], ctxps[hi][64:65, :]
                            )
                        else:
                            nc.vector.tensor_copy(
                                ztmp_z[64:65, h, :], ctxps[hi][64:65, :]
                            )
                    for hi, h in enumerate(G):
                        po = (h % 2) * 64
                        if tail and hi == 0:
                            nc.scalar.copy(
                                ctx_sb[po:po + 64, gg, :], ctxps[hi][0:64, :]
                            )
                        else:
                            nc.vector.tensor_copy(
                                ctx_sb[po:po + 64, gg, :], ctxps[hi][0:64, :]
                            )

                def s_transp(which=(0, 1)):
                    if "zt_g" not in state:
                        state["zt_g"] = mmps.tile(
                            [P, 512], f32, tag="mm", name=f"zt{gg}"
                        )
                    zt_g = state["zt_g"]
                    for hi in which:
                        h = G[hi]
                        for qc in range(4):
                            nc.tensor.transpose(
                                zt_g[:, hi * 4 + qc:hi * 4 + qc + 1],
                                ztmp_z[64:65, h, qc * P:(qc + 1) * P],
                                ident_f[64:65, 64:65],
                            )

                def s_recip():
                    nc.vector.tensor_copy(
                        rz_t[:, gg * 8:gg * 8 + 8], state["zt_g"][:, 0:8]
                    )
                    nc.vector.reciprocal(
                        rz_t[:, gg * 8:gg * 8 + 8], rz_t[:, gg * 8:gg * 8 + 8]
                    )

                def s_fc(qc):
                    # fc split per head (row groups 0/64 run concurrently);
                    # 1/Z applied to the [q, F] outputs as per-partition
                    # scalar. At the tail the score psum banks are free, so
                    # the fc pairs draw from them (deeper pipelining than the
                    # 2-buffer mm pool allows).
                    def emit():
                        pss = []
                        if tail:
                            ps2 = spsum.tile(
                                [P, 2 * QB], f32, tag="sc", name=f"pso{gg}{qc}"
                            )
                            pss = [ps2[:, 0:F], ps2[:, QB:QB + F]]
                        else:
                            pss = [
                                mmps.tile(
                                    [P, 512], f32, tag="mm",
                                    name=f"pso{gg}{qc}{hi}",
                                )[:, :F]
                                for hi in range(2)
                            ]
                        for hi in range(2):
                            po = hi * 64
                            nc.tensor.matmul(
                                pss[hi],
                                ctx_sb[po:po + 64, gg, qc * P:(qc + 1) * P],
                                wfc_sb[po:po + 64, gg, :],
                                start=True,
                                stop=True,
                            )
                        rz0 = rz_t[:, gg * 8 + qc:gg * 8 + qc + 1]
                        rz1 = rz_t[:, gg * 8 + 4 + qc:gg * 8 + 4 + qc + 1]
                        if gg == 0:
                            t0 = opool.tile([P, F], f32, tag="x", name=f"t0{qc}")
                            nc.vector.scalar_tensor_tensor(
                                t0, pss[0], rz0, res_t[:, qc, :],
                                op0=ALU.mult, op1=ALU.add,
                            )
                            nc.vector.scalar_tensor_tensor(
                                xacc[:, qc, :], pss[1], rz1, t0,
                                op0=ALU.mult, op1=ALU.add,
                            )
                        elif tail:
                            # scalar + gpsimd are idle at the tail: spread the
                            # per-qc chain so the vector engine stops binding
                            t0 = opool.tile([P, F], f32, tag="x", name=f"u0{qc}")
                            nc.scalar.mul(t0, pss[0], rz0)
                            x1 = opool.tile([P, F], f32, tag="x", name=f"v1{qc}")
                            nc.vector.scalar_tensor_tensor(
                                x1, pss[1], rz1, xacc[:, qc, :],
                                op0=ALU.mult, op1=ALU.add,
                            )
                            x_t = opool.tile([P, F], f32, tag="x", name=f"x{qc}")
                            nc.gpsimd.tensor_add(x_t, x1, t0)
                            st = opool.tile([P, 6], f32, tag="st", name=f"st{qc}")
                            nc.vector.bn_stats(st, x_t)
                            mv = opool.tile([P, 2], f32, tag="mv", name=f"mv{qc}")
                            nc.vector.bn_aggr(mv, st)
                            nc.scalar.activation(
                                mv[:, 1:2], mv[:, 1:2], AF.Sqrt,
                                bias=eps_t, scale=1.0,
                            )
                            nc.vector.reciprocal(mv[:, 1:2], mv[:, 1:2])
                            # negmb = -(mean * rstd); normalize on the scalar
                            # engine: out = Identity(x * rstd + negmb)
                            nc.vector.tensor_scalar(
                                mv[:, 0:1],
                                mv[:, 0:1],
                                mv[:, 1:2],
                                -1.0,
                                op0=ALU.mult,
                                op1=ALU.mult,
                            )
                            nc.scalar.activation(
                                o_sb[:, qc, :], x_t, AF.Identity,
                                bias=mv[:, 0:1], scale=mv[:, 1:2],
                            )
                            # ship each 128-query chunk as soon as it's done,
                            # alternating the two DMA rings
                            if qc % 2 == 0:
                                nc.sync.dma_start(
                                    out[:, qc:qc + 1, :], o_sb[:, qc:qc + 1, :]
                                )
                            else:
                                nc.scalar.dma_start(
                                    out[:, qc:qc + 1, :], o_sb[:, qc:qc + 1, :]
                                )
                    return emit

                return [s_copies, s_transp, s_recip,
                        s_fc(0), s_fc(1), s_fc(2), s_fc(3)]

            # ---- group 0: projections interleaved with its attention ----
            G0, G1 = (0, 1), (2, 3)
            ctxps0 = [
                cpsum.tile([P, QB], f32, tag="ctxp", name=f"ctxp{hh}")
                for hh in G0
            ]
            # HAM pre-warm: ~3.8us of dummy f16 matmuls in the input-DMA
            # shadow so the PE clock-gate opens (1.2 -> 2.4 GHz) before the
            # real stream begins. Results land in ctxps0[0] and are discarded
            # by the start=True of PV kc 0.
            for i in range(34):
                nc.tensor.matmul(
                    ctxps0[0][:, 0:P], dummy16, dummy16,
                    start=True, stop=True,
                )
            # head ordering: G0 (heads 0,1) only reads the g=0 halves of
            # Q^T/K^T and heads 0,1 of V, so only those are projected up
            # front; all g=1/h23 projections are dripped in later, ahead of
            # the G1 phase. This gets the first exps issued ~2us earlier.
            project_kt(0, 0)
            project_kt(0, 1)
            project_qt(0)
            project_qt(1)
            pt0 = attn_sc(G0, 0)
            pt1 = attn_sc(G0, 1)
            project_v(0)
            pt2 = attn_sc(G0, 2)
            attn_pv(G0, ctxps0, 0, pt0)
            pt3 = attn_sc(G0, 3)
            attn_pv(G0, ctxps0, 1, pt1)
            attn_pv(G0, ctxps0, 2, pt2)
            attn_pv(G0, ctxps0, 3, pt3)

            def proj(*specs):
                def emit():
                    for f, *a in specs:
                        f(*a)
                return emit
            proj_drip = {
                3: proj((project_kt, 1, 0), (project_kt, 1, 1),
                        (project_v, 1)),
                6: proj((project_kt, 2, 0), (project_kt, 2, 1)),
                7: proj((project_v, 2),),
                10: proj((project_kt, 3, 0), (project_kt, 3, 1)),
                11: proj((project_v, 3),),
            }
            proj_drip[3]()
            for kc in range(4, KC):
                attn_kc(G0, ctxps0, kc)
                if kc in proj_drip:
                    proj_drip[kc]()
            steps0 = epilogue_steps(G0, ctxps0)
            steps0[0]()  # ctx/z copies (DVE only, releases ctxps slots)

            # ---- group 1: attention with group-0 epilogue drip-fed in ----
            ctxps1 = [
                cpsum.tile([P, QB], f32, tag="ctxp", name=f"ctxp{hh}")
                for hh in G1
            ]
            drip = {1: lambda: steps0[1]((0,)), 2: lambda: steps0[1]((1,)),
                    4: steps0[2], 6: steps0[3], 8: steps0[4],
                    10: steps0[5], 12: steps0[6]}
            for kc in range(KC):
                attn_kc(G1, ctxps1, kc)
                if kc in drip:
                    drip[kc]()
            # keep the PE busy while the last exps drain so the HAM gate
            # stays open for the epilogue transposes + fc matmuls
            steps1 = epilogue_steps(G1, ctxps1, tail=True)
            steps1[0]()  # ctx/z copies
            warm_ps = spsum.tile([P, 2 * QB], f32, tag="sc", name="warmtail")
            for i in range(8):
                nc.tensor.matmul(
                    warm_ps[:, 0:P], dummy16, dummy16,
                    start=True, stop=True,
                )
            for step in steps1[1:]:
                step()

    nc.compile()
    return nc


def get_nc():
    if "nc" not in _CACHE:
        _CACHE["nc"] = _build_program()
    return _CACHE["nc"]


def make_in_maps(input_Q, input_K, input_V, W_Q, W_K, W_V, W_fc):
    import ml_dtypes

    f8 = ml_dtypes.float8_e4m3
    c16 = lambda a: np.ascontiguousarray(
        np.asarray(a, dtype=np.float32), dtype=np.float16
    )
    c8 = lambda a: np.clip(np.asarray(a, dtype=np.float32), -240, 240).astype(f8)
    # pack an [in, out]-style matrix to SBUF layout [p, c, out]
    pk_w = lambda w: np.asarray(w, np.float32).reshape(2, P, -1).transpose(1, 0, 2)
    # pack an activation block X [seq, F] to X^T SBUF layout [p, c, seq]
    pk_t = lambda x: c8(np.asarray(x, np.float32).T.reshape(2, P, -1).transpose(1, 0, 2))
    # pack a rolled key/value matrix [2048, F] to per-block X^T [nb, p, c, 512]
    pk_x = lambda x: np.asarray(x, np.float32).reshape(4, 512, 2, P).transpose(0, 3, 2, 1)
    e01t, e25t = _gauss_tables()
    e01 = np.ascontiguousarray(e01t.transpose(1, 0, 2))
    e25 = np.ascontiguousarray(e25t.transpose(1, 0, 2))
    e01_neutral = np.ones_like(e01)
    # unscaled W_Q; 1/sqrt(d_k) folded into the exp scale
    wq = c8(pk_w(W_Q))
    wk = c8(pk_w(W_K))
    wv8 = c8(pk_w(W_V))
    # wfr: [P, 6, F] f16 = W_fc (2 c-chunks) ++ residual (4 seq chunks)
    wfc_p = np.asarray(W_fc, np.float32).reshape(2, P, F).transpose(1, 0, 2)
    in_maps = []
    for c in range(N_CORES):
        b, qb = divmod(c, 4)
        q0 = qb * QB
        r = (q0 - 256) % S
        xq_blk = np.asarray(input_Q[b][q0:q0 + QB], np.float32)
        xk_rot = np.roll(np.asarray(input_K[b], np.float32), -r, axis=0)
        xv_rot = np.roll(np.asarray(input_V[b], np.float32), -r, axis=0)
        xkv = c8(np.stack([pk_x(xk_rot), pk_x(xv_rot)], axis=2))
        wfr = c16(np.concatenate(
            [wfc_p, xq_blk.reshape(4, P, F).transpose(1, 0, 2)], axis=1
        ))
        in_maps.append({
            "xqt": pk_t(xq_blk),
            "wq8": wq,
            "wk8": wk,
            "wv8": wv8,
            "xkv": xkv,
            "wfr": wfr,
            "e01": e01_neutral if q0 == 0 else e01,
            "e25": e25,
        })
    return in_maps


def kernel(input_Q, input_K, input_V, W_Q, W_K, W_V, W_fc, attn_mask=None):
    from concourse.bass_utils import run_bass_kernel_spmd

    nc = get_nc()
    in_maps = make_in_maps(input_Q, input_K, input_V, W_Q, W_K, W_V, W_fc)
    res = run_bass_kernel_spmd(nc, in_maps, core_ids=list(range(N_CORES)))
    out = np.empty((B, S, F), dtype=np.float32)
    for c in range(N_CORES):
        b, qb = divmod(c, 4)
        o = res.results[c]["out"]
        out[b, qb * QB:(qb + 1) * QB, :] = o.transpose(1, 0, 2).reshape(QB, F)
    return out
